# revision 19
# baseline (speedup 1.0000x reference)
"""Trainium2 Bass kernel for nn_BiInteraction (bilinear atom/protein attention).

Strategy (per sharding hint): shard the 64 molecules across 8 NeuronCores
(8 molecules per core), replicate all weights. Each core computes its 8
molecules end-to-end (scores, both attention pools, MLP); the host
concatenates the per-core [8, 1] outputs into [64, 1].

Per-core dataflow (B=8 molecules, L padded 1000->1024, 64 atom slots/mol):
  - transpose atom embeds + protein embeds via PE (D must sit on partitions
    for the score matmuls)
  - qT = Wa^T @ A^T (one matmul)
  - scores per 128-row L-chunk, layout [l, atom-slot]: mask bias is injected
    pre-tanh via a K=8 one-hot matmul accumulated in PSUM
  - tanh on ACT; protein-side segment-max = windowed free-dim reduce;
    atom-side max = tensor_tensor max trees (chunks, then partitions)
  - both attention pools are computed transposed ([d, mol] columns) so the
    2-layer MLP runs with molecule columns as the moving operand
"""

import os
import sys
from contextlib import ExitStack

import numpy as np

for _p in ("/opt/trn_rl_repo", "/root/.axon_site/_ro/trn_rl_repo"):
    if os.path.isdir(_p) and _p not in sys.path:
        sys.path.insert(0, _p)

import concourse.bass as bass
import concourse.tile as tile
from concourse import mybir
from concourse.bass_utils import run_bass_kernel_spmd
from concourse.masks import make_identity

F32 = mybir.dt.float32
NEG = -9e15

NDEV = 8
B = 64          # total molecules
BL = B // NDEV  # molecules per core
L = 1000
LP = 1024       # padded L
NCH = LP // 128  # 8 L-chunks of 128
D = 128
ASLOT = 64      # atom slots per molecule (padded)
NA = BL * ASLOT  # 512 atom slots per core
H1, H2 = 512, 256


def _split_multi_waits(nc):
    """This walrus build rejects >1 sync-wait per instruction (any class).
    Hoist extra waits onto same-engine NOPs inserted just before each
    offending instruction (equivalent semantics: the engine blocks at the
    NOP instead of at the instruction itself)."""
    k = 0
    for f in nc.m.functions:
        for bb in f.blocks:
            rebuilt = []
            changed = False
            for ins in bb.instructions:
                si = ins.sync_info
                waits = list(si.on_wait) if si and si.on_wait else []
                if len(waits) > 1:
                    changed = True
                    for w in waits[:-1]:
                        k += 1
                        rebuilt.append(mybir.InstNoOp(
                            name=f"waitnop-{k}",
                            sync_info=mybir.SyncInfo(on_wait=[w], on_update=[]),
                            bass_nofuse=True,
                            engine=ins.engine,
                        ))
                    si.on_wait = [waits[-1]]
                rebuilt.append(ins)
            if changed:
                bb.instructions[:] = rebuilt


def _build_program(counts, loop_n=None):
    """counts: per-molecule atom counts (length B), already asserted <= ASLOT."""
    nc = bass.Bass()

    a_ext = nc.declare_dram_parameter("a", [NA, D], F32, isOutput=False)
    p_ext = nc.declare_dram_parameter("p", [BL, LP, D], F32, isOutput=False)
    bias_ext = nc.declare_dram_parameter("bias", [BL, LP], F32, isOutput=False)
    valid_ext = nc.declare_dram_parameter("valid", [NA], mybir.dt.int32, isOutput=False)
    oneh_ext = nc.declare_dram_parameter("onehot", [BL, NA], F32, isOutput=False)
    wa_ext = nc.declare_dram_parameter("wa", [D, D], F32, isOutput=False)
    w1_ext = nc.declare_dram_parameter("w1", [2 * D, H1], F32, isOutput=False)
    b1_ext = nc.declare_dram_parameter("b1", [H1], F32, isOutput=False)
    w2_ext = nc.declare_dram_parameter("w2", [H1, H2], F32, isOutput=False)
    b2_ext = nc.declare_dram_parameter("b2", [H2], F32, isOutput=False)
    wo_ext = nc.declare_dram_parameter("wo", [H2, 1], F32, isOutput=False)
    bo_ext = nc.declare_dram_parameter("bo", [1], F32, isOutput=False)
    y_ext = nc.declare_dram_parameter("y", [BL, 1], F32, isOutput=True)

    uniform = len(set(int(c) for c in counts)) == 1
    cnt0 = int(counts[0])

    with ExitStack() as ctx:
        tc = ctx.enter_context(tile.TileContext(nc))
        if loop_n is not None:
            ctx.enter_context(tc.For_i(0, loop_n, 1))
        P = ctx.enter_context(tc.tile_pool(name="persist", bufs=1))

        def pt(shape, tag):
            return P.tile(shape, F32, tag=tag, name=tag)

        # ---- constants / weights into SBUF ----
        ident = pt([128, 128], "ident")
        make_identity(nc, ident)
        ones_col = pt([128, 1], "ones_col")
        nc.vector.memset(ones_col, 1.0)
        ones_row = pt([1, 128], "ones_row")
        nc.gpsimd.memset(ones_row, 1.0)

        wa_sb = pt([D, D], "wa")
        nc.scalar.dma_start(out=wa_sb, in_=wa_ext[:, :])
        w1_sb = [pt([128, H1], f"w1_{h}") for h in range(2)]
        for h in range(2):
            nc.scalar.dma_start(out=w1_sb[h], in_=w1_ext[128 * h:128 * (h + 1), :])
        w2_sb = [pt([128, H2], f"w2_{k}") for k in range(4)]
        for k in range(4):
            nc.scalar.dma_start(out=w2_sb[k], in_=w2_ext[128 * k:128 * (k + 1), :])
        wo_sb = pt([128, 2], "wo")
        nc.scalar.dma_start(out=wo_sb, in_=wo_ext.rearrange("(c p) one -> p (c one)", p=128))
        b1t_sb = pt([128, 4], "b1t")
        nc.scalar.dma_start(out=b1t_sb, in_=b1_ext.rearrange("(c p) -> p c", p=128))
        b2t_sb = pt([128, 2], "b2t")
        nc.scalar.dma_start(out=b2t_sb, in_=b2_ext.rearrange("(c p) -> p c", p=128))
        bo_sb = pt([BL, 1], "bo")
        bo_ap = bo_ext[:]
        nc.scalar.dma_start(
            out=bo_sb,
            in_=bass.AP(tensor=bo_ap.tensor, offset=bo_ap.offset, ap=[[0, BL], [1, 1]]),
        )
        bias_sb = pt([BL, LP], "bias")
        nc.scalar.dma_start(out=bias_sb, in_=bias_ext[:, :])
        oneh_sb = pt([BL, NA], "oneh")
        nc.scalar.dma_start(out=oneh_sb, in_=oneh_ext[:, :])
        valid_sb = P.tile([128, NA // 128], mybir.dt.int32, tag="valid", name="valid")
        nc.scalar.dma_start(out=valid_sb, in_=valid_ext.rearrange("(c p) -> p c", p=128))

        a_sb = [pt([128, D], f"a_{t}") for t in range(NA // 128)]
        for t in range(NA // 128):
            nc.sync.dma_start(out=a_sb[t], in_=a_ext[128 * t:128 * (t + 1), :])

        pnat = {}
        for b in range(BL):
            for c in range(NCH):
                pnat[b, c] = pt([128, D], f"pnat_{b}_{c}")
                nc.sync.dma_start(out=pnat[b, c], in_=p_ext[b, 128 * c:128 * (c + 1), :])

        # ---- phase 1+2: transposes and qT ----
        pt_sb = [pt([128, LP], f"ptr_{b}") for b in range(BL)]
        at_sb = pt([128, NA], "at")
        qt_sb = pt([128, NA], "qt")

        copy_engines = [nc.vector.tensor_copy, nc.scalar.copy]

        with tc.tile_pool(name="psum1", bufs=1, space="PSUM") as ps1:
            at_ps = ps1.tile([128, NA], F32, tag="m512", name="at_ps")
            for t in range(NA // 128):
                nc.tensor.transpose(at_ps[:, 128 * t:128 * (t + 1)], a_sb[t], ident)
            nc.vector.tensor_copy(at_sb, at_ps)
            qt_ps = ps1.tile([128, NA], F32, tag="m512b", name="qt_ps")
            nc.tensor.matmul(qt_ps, lhsT=wa_sb, rhs=at_sb, start=True, stop=True)
            nc.scalar.copy(qt_sb, qt_ps)

            for b in range(BL):
                ptp = ps1.tile([128, LP], F32, tag="ptps", bufs=2, name="ptp")
                for c in range(NCH):
                    nc.tensor.transpose(ptp[:, 128 * c:128 * (c + 1)], pnat[b, c], ident)
                for h in range(2):
                    copy_engines[(2 * b + h) % 2](
                        pt_sb[b][:, 512 * h:512 * (h + 1)], ptp[:, 512 * h:512 * (h + 1)]
                    )

            # ---- phase 3: scores per L-chunk ----
            t_sb = [pt([128, NA], f"t_{c}") for c in range(NCH)]
            wp_cols = [pt([128, BL], f"wpc_{c}") for c in range(NCH)]
            for c in range(NCH):
                s_ps = ps1.tile([128, NA], F32, tag="scps", bufs=2, name="s_ps")
                nc.tensor.matmul(
                    s_ps, lhsT=bias_sb[:, 128 * c:128 * (c + 1)], rhs=oneh_sb,
                    start=True, stop=False, skip_group_check=True,
                )
                for b in range(BL):
                    cb = int(counts[b])
                    nc.tensor.matmul(
                        s_ps[:, ASLOT * b:ASLOT * b + cb],
                        lhsT=pt_sb[b][:, 128 * c:128 * (c + 1)],
                        rhs=qt_sb[:, ASLOT * b:ASLOT * b + cb],
                        start=False, stop=(b == BL - 1), skip_group_check=True,
                    )
                nc.scalar.activation(t_sb[c], s_ps, mybir.ActivationFunctionType.Tanh)
                if uniform:
                    nc.vector.reduce_max(
                        wp_cols[c],
                        t_sb[c].rearrange("p (b s) -> p b s", b=BL)[:, :, 0:cnt0],
                        axis=mybir.AxisListType.X,
                    )
                else:
                    for b in range(BL):
                        nc.vector.reduce_max(
                            wp_cols[c][:, b:b + 1],
                            t_sb[c][:, ASLOT * b:ASLOT * b + int(counts[b])],
                            axis=mybir.AxisListType.X,
                        )

        # ---- phase 4: atom-side max trees -> Wc ----
        m1 = [pt([128, NA], f"m1_{j}") for j in range(4)]
        m2 = [pt([128, NA], f"m2_{j}") for j in range(2)]
        m3 = pt([128, NA], "m3")
        for j in range(4):
            nc.vector.tensor_max(m1[j], t_sb[2 * j], t_sb[2 * j + 1])
        nc.vector.tensor_max(m2[0], m1[0], m1[1])
        nc.vector.tensor_max(m2[1], m1[2], m1[3])
        nc.vector.tensor_max(m3, m2[0], m2[1])
        # partition-dim max: PE-transpose m3 chunks to put partitions on the
        # free dim, then windowed reduce; lands directly in the [p, c] grid
        # layout (slot = p + 128c)
        wccv = pt([128, NA // 128], "wccv")

        # ---- phase 5: protein softmax + pools + MLP ----
        wp_sb = pt([BL, LP], "wp")
        wpm_sb = pt([BL, LP], "wpm")
        e_sb = pt([BL, LP], "e")
        ap_sb = pt([BL, LP], "ap")
        rmax = pt([BL, 1], "rmax")
        rmax_neg = pt([BL, 1], "rmax_neg")
        sume = pt([BL, 1], "sume")
        rsum = pt([BL, 1], "rsum")
        apc_sb = [pt([128, BL], f"apc_{c}") for c in range(NCH)]
        sc_row = pt([1, BL], "sc_row")
        rsc = pt([1, BL], "rsc")
        hta = pt([128, BL], "hta")
        htp = pt([128, BL], "htp")
        h1t = [pt([128, BL], f"h1t_{j}") for j in range(4)]
        h2t = [pt([128, BL], f"h2t_{j}") for j in range(2)]
        y_sb = pt([BL, 1], "y")

        with tc.tile_pool(name="psum2", bufs=1, space="PSUM") as ps2:
            m_grid = pt([128, NA // 128], "m_grid")
            for j in range(NA // 128):
                mt_ps = ps2.tile([128, 128], F32, tag="small", bufs=4, name="mt_ps")
                nc.tensor.transpose(mt_ps, m3[:, 128 * j:128 * (j + 1)], ident)
                nc.vector.reduce_max(
                    m_grid[:, j:j + 1], mt_ps, axis=mybir.AxisListType.X,
                )
            wc_grid = pt([128, NA // 128], "wc_grid")
            nc.scalar.activation(wc_grid, m_grid, mybir.ActivationFunctionType.Exp)
            nc.vector.memset(wccv, 0.0)
            nc.vector.copy_predicated(out=wccv, mask=valid_sb, data=wc_grid)

            wpt_ps = ps2.tile([BL, LP], F32, tag="wpt", name="wpt_ps")
            for c in range(NCH):
                nc.tensor.transpose(
                    wpt_ps[:, 128 * c:128 * (c + 1)], wp_cols[c], ident
                )
            nc.scalar.copy(wp_sb, wpt_ps)
            nc.vector.tensor_add(wpm_sb, wp_sb, bias_sb)
            nc.vector.reduce_max(rmax, wpm_sb, axis=mybir.AxisListType.X)
            nc.scalar.mul(rmax_neg, rmax, -1.0)
            nc.scalar.activation(
                e_sb, wpm_sb, mybir.ActivationFunctionType.Exp,
                bias=rmax_neg, scale=1.0, accum_out=sume,
            )
            nc.vector.reciprocal(rsum, sume)
            nc.scalar.mul(ap_sb, e_sb, rsum)

            for c in range(NCH):
                apt_ps = ps2.tile([128, BL], F32, tag="apt", bufs=2, name="apt_ps")
                nc.tensor.transpose(
                    apt_ps, ap_sb[:, 128 * c:128 * (c + 1)], ident[0:BL, 0:BL]
                )
                copy_engines[c % 2](apc_sb[c], apt_ps)

            # Sc (atom-weight normalizers) via tiny matmuls
            sc_ps = ps2.tile([1, BL], F32, tag="small", bufs=4, name="sc_ps")
            for b in range(BL):
                po = (b % 2) * ASLOT
                tcol = b // 2
                nc.tensor.matmul(
                    sc_ps[:, b:b + 1],
                    lhsT=wccv[po:po + ASLOT, tcol:tcol + 1],
                    rhs=ones_col[po:po + ASLOT, :],
                    start=True, stop=True,
                )
            nc.scalar.copy(sc_row, sc_ps)
            nc.vector.reciprocal(rsc, sc_row)

            # atom pool (transposed): columns = molecules
            apool_ps = ps2.tile([128, BL], F32, tag="small", bufs=4, name="apool_ps")
            for b in range(BL):
                po = (b % 2) * ASLOT
                tcol = b // 2
                nc.tensor.matmul(
                    apool_ps[:, b:b + 1],
                    lhsT=a_sb[b // 2][po:po + ASLOT, :],
                    rhs=wccv[po:po + ASLOT, tcol:tcol + 1],
                    start=True, stop=True,
                )
            sg_ps = ps2.tile([128, BL], F32, tag="small", bufs=4, name="sg_ps")
            nc.tensor.matmul(sg_ps, lhsT=ones_row, rhs=rsc, start=True, stop=True)
            apool_sb = pt([128, BL], "apool_sb")
            nc.scalar.copy(apool_sb, apool_ps)
            nc.vector.tensor_mul(hta, apool_sb, sg_ps)

            # protein pool (transposed)
            pp_ps = ps2.tile([128, BL], F32, tag="small", bufs=4, name="pp_ps")
            for b in range(BL):
                for c in range(NCH):
                    nc.tensor.matmul(
                        pp_ps[:, b:b + 1],
                        lhsT=pnat[b, c],
                        rhs=apc_sb[c][:, b:b + 1],
                        start=(c == 0), stop=(c == NCH - 1),
                    )
            nc.scalar.copy(htp, pp_ps)

            # ---- MLP ----
            for j in range(4):
                h1_ps = ps2.tile([128, BL], F32, tag="small", bufs=4, name="h1_ps")
                nc.tensor.matmul(
                    h1_ps, lhsT=w1_sb[0][:, 128 * j:128 * (j + 1)], rhs=hta,
                    start=True, stop=False,
                )
                nc.tensor.matmul(
                    h1_ps, lhsT=w1_sb[1][:, 128 * j:128 * (j + 1)], rhs=htp,
                    start=False, stop=True,
                )
                nc.scalar.activation(
                    h1t[j], h1_ps, mybir.ActivationFunctionType.Relu,
                    bias=b1t_sb[:, j:j + 1],
                )
            for j2 in range(2):
                h2_ps = ps2.tile([128, BL], F32, tag="small", bufs=4, name="h2_ps")
                for kc in range(4):
                    nc.tensor.matmul(
                        h2_ps, lhsT=w2_sb[kc][:, 128 * j2:128 * (j2 + 1)],
                        rhs=h1t[kc], start=(kc == 0), stop=(kc == 3),
                    )
                nc.scalar.activation(
                    h2t[j2], h2_ps, mybir.ActivationFunctionType.Relu,
                    bias=b2t_sb[:, j2:j2 + 1],
                )
            y_ps = ps2.tile([BL, 1], F32, tag="small", bufs=4, name="y_ps")
            for j2 in range(2):
                nc.tensor.matmul(
                    y_ps, lhsT=h2t[j2], rhs=wo_sb[:, j2:j2 + 1],
                    start=(j2 == 0), stop=(j2 == 1),
                )
            nc.scalar.activation(
                y_sb, y_ps, mybir.ActivationFunctionType.Identity, bias=bo_sb
            )
            nc.sync.dma_start(out=y_ext[:, :], in_=y_sb)

    _split_multi_waits(nc)
    return nc


_CACHE = {}


def _prep(atom_embed, protSeq_embed, atom_splits, protSeq_len,
          Wa, W1, b1, W2, b2, Wo, bo):
    atom_embed = np.asarray(atom_embed, dtype=np.float32)
    protSeq_embed = np.asarray(protSeq_embed, dtype=np.float32)
    atom_splits = np.asarray(atom_splits, dtype=np.int64)
    protSeq_len = np.asarray(protSeq_len, dtype=np.int64)
    Wa = np.asarray(Wa, dtype=np.float32)
    W1 = np.asarray(W1, dtype=np.float32)
    b1 = np.asarray(b1, dtype=np.float32)
    W2 = np.asarray(W2, dtype=np.float32)
    b2 = np.asarray(b2, dtype=np.float32)
    Wo = np.asarray(Wo, dtype=np.float32)
    bo = np.asarray(bo, dtype=np.float32)

    counts = np.bincount(atom_splits, minlength=B)
    assert counts.max() <= ASLOT, f"molecule with {counts.max()} atoms > {ASLOT} slots"
    offsets = np.concatenate([[0], np.cumsum(counts)])

    key = tuple(counts.tolist())
    if key not in _CACHE:
        _CACHE[key] = _build_program(counts)
    nc = _CACHE[key]

    lp_iota = np.arange(LP)
    in_maps = []
    for dev in range(NDEV):
        mols = range(dev * BL, (dev + 1) * BL)
        a_dev = np.zeros((NA, D), np.float32)
        valid_dev = np.zeros((NA,), np.int32)
        for bl, m in enumerate(mols):
            cnt = counts[m]
            a_dev[ASLOT * bl:ASLOT * bl + cnt] = atom_embed[offsets[m]:offsets[m + 1]]
            valid_dev[ASLOT * bl:ASLOT * bl + cnt] = 1
        p_dev = np.zeros((BL, LP, D), np.float32)
        p_dev[:, :L, :] = protSeq_embed[dev * BL:(dev + 1) * BL]
        lens = protSeq_len[dev * BL:(dev + 1) * BL]
        bias_dev = np.where(lp_iota[None, :] < lens[:, None], 0.0, NEG).astype(np.float32)
        onehot = (lp_iota[:NA][None, :] // ASLOT == np.arange(BL)[:, None]).astype(np.float32)
        in_maps.append({
            "a": a_dev, "p": p_dev, "bias": bias_dev, "valid": valid_dev,
            "onehot": onehot, "wa": Wa, "w1": W1, "b1": b1, "w2": W2,
            "b2": b2, "wo": Wo, "bo": bo,
        })

    return nc, in_maps


def kernel(**inputs):
    nc, in_maps = _prep(**inputs)
    res = run_bass_kernel_spmd(nc, in_maps, list(range(NDEV)))
    y = np.concatenate([res.results[i]["y"] for i in range(NDEV)], axis=0)
    return y.astype(np.float32)


def run_traced(**inputs):
    """Run with NTFF profiling; returns (y, BassKernelResults)."""
    nc, in_maps = _prep(**inputs)
    res = run_bass_kernel_spmd(nc, in_maps, list(range(NDEV)), trace=True)
    y = np.concatenate([res.results[i]["y"] for i in range(NDEV)], axis=0)
    return y.astype(np.float32), res


def time_kernel(k_lo=4, k_hi=36, reps=5, **inputs):
    """On-device loop timing: per-iteration ns from the wall-clock delta
    between a k_hi-iteration NEFF and a k_lo-iteration NEFF."""
    import time as _t
    counts = np.bincount(np.asarray(inputs["atom_splits"], np.int64), minlength=B)
    _, in_maps = _prep(**inputs)  # builds base program + in_maps
    times = {}
    for k in (k_lo, k_hi):
        nck = _build_program(counts, loop_n=k)
        run_bass_kernel_spmd(nck, in_maps, list(range(NDEV)))  # warm (compile+run)
        best = float("inf")
        for _ in range(reps):
            t0 = _t.perf_counter()
            run_bass_kernel_spmd(nck, in_maps, list(range(NDEV)))
            best = min(best, _t.perf_counter() - t0)
        times[k] = best
    ns = (times[k_hi] - times[k_lo]) / (k_hi - k_lo) * 1e9
    return ns, times
